# revision 34
# baseline (speedup 1.0000x reference)
"""Fused ARFlow kernel for Trainium2 (8 NeuronCores, data-parallel over batch).

Reference computes three causal K=3 convs (64->256->256->128 ch) with NO
nonlinearity between them, then z = exp(alpha*tanh(ls)+beta)*x + tt.
The convs are linear, so they compose on the host into a single causal K=7
conv (64->128 ch) with an effective bias, exact for t>=4; an x-independent
(weights-only) correction D fixes outputs t<4 where the reference's
zero-padding of *biased* intermediates differs from the composition.

Device kernel per core (4 samples): for each 1024-wide time tile,
  - load x with a 7-tap halo into a [128, TS+8] SBUF tile: partitions 0-63
    hold x shifted by 7, partitions 64-127 hold x shifted by 6 (tap-pair
    packing -> 4 fp16 matmuls of contraction 128 instead of 7 of 64),
  - 8 fp16 matmuls (two 512-col halves) accumulate the conv into PSUM,
  - ScalarE: th = tanh(ps[0:64] + b_ls); e = exp(th*alpha + beta),
  - VectorE: zm = e * x;  z = (ps[64:128] + b_hi) + zm,
  - DMA z back.
"""

import numpy as np

import concourse.bacc as bacc
import concourse.bass as bass
import concourse.mybir as mybir
import concourse.tile as tile
from concourse.bass_utils import run_bass_kernel_spmd

N_CORES = 8
B, C, T = 32, 64, 8192
NS = B // N_CORES          # samples per core
TS = 1024                  # time-tile width (multiple of 512)
O = 128                    # output channels (2C)

F32 = mybir.dt.float32
F32R = mybir.dt.float32r
F16 = mybir.dt.float16
USE_XF = True


# ---------------------------------------------------------------- host math

def _compose(w_in, b_in, w_mid, b_mid, w_out, b_out):
    """W_eff (128, 64, 7), b_eff (128,), D (128, 4)."""
    w_in = np.asarray(w_in, np.float64)
    w_mid = np.asarray(w_mid, np.float64)
    w_out = np.asarray(w_out, np.float64)
    b_in = np.asarray(b_in, np.float64)
    b_mid = np.asarray(b_mid, np.float64)
    b_out = np.asarray(b_out, np.float64)
    H = w_in.shape[0]

    w12 = np.zeros((w_mid.shape[0], w_in.shape[1], 5))
    for i in range(3):
        for j in range(3):
            w12[:, :, i + j] += w_mid[:, :, j] @ w_in[:, :, i]
    w_eff = np.zeros((w_out.shape[0], w_in.shape[1], 7))
    for i in range(5):
        for l in range(3):
            w_eff[:, :, i + l] += w_out[:, :, l] @ w12[:, :, i]

    b_eff = w_out.sum(2) @ (w_mid.sum(2) @ b_in + b_mid) + b_out

    # boundary correction: reference chain on x=0, minus steady-state b_eff
    Tz = 12
    rs1 = np.zeros((H, Tz)) + b_in[:, None]
    rs2 = np.zeros((w_mid.shape[0], Tz))
    for t in range(Tz):
        acc = b_mid.copy()
        for j in range(3):
            tau = t - 2 + j
            if tau >= 0:
                acc = acc + w_mid[:, :, j] @ rs1[:, tau]
        rs2[:, t] = acc
    rs3 = np.zeros((w_out.shape[0], Tz))
    for t in range(Tz):
        acc = b_out.copy()
        for l in range(3):
            tau = t - 2 + l
            if tau >= 0:
                acc = acc + w_out[:, :, l] @ rs2[:, tau]
        rs3[:, t] = acc
    D = rs3[:, 0:4] - b_eff[:, None]
    return (w_eff.astype(np.float32), b_eff.astype(np.float32),
            D.astype(np.float32))


def _pack_weights(w_eff):
    """Stationary lhsT tiles (4, 128, 128): WT[m][c, o] = W_eff[o, c, 2m],
    WT[m][64+c, o] = W_eff[o, c, 2m+1] (tap 7 = zero)."""
    WT = np.zeros((4, 128, 128), np.float32)
    for m in range(4):
        WT[m, 0:64, :] = w_eff[:, :, 2 * m].T
        if 2 * m + 1 <= 6:
            WT[m, 64:128, :] = w_eff[:, :, 2 * m + 1].T
    return WT


# ------------------------------------------------------------- device build

def build_nc(ns=NS, t_len=T, ts=TS):
    assert ts % 512 == 0 and t_len % ts == 0
    nc = bacc.Bacc("TRN2", target_bir_lowering=False, debug=False,
                   num_devices=N_CORES)
    xs = nc.dram_tensor("xs", (ns, C, t_len), F16, kind="ExternalInput").ap()
    xf = nc.dram_tensor("xf", (ns, C, t_len), F32, kind="ExternalInput").ap()
    wt = nc.dram_tensor("wt", (4, 128, 128), F16, kind="ExternalInput").ap()
    # boundary correction D.T (applied to t=0..3 via one-hot indicator rhs)
    bdd = nc.dram_tensor("bdd", (4, 128), F16, kind="ExternalInput").ap()
    bind = nc.dram_tensor("bind", (4, 512), F16, kind="ExternalInput").ap()
    zpad = nc.dram_tensor("zpad", (128, 8), F16, kind="ExternalInput").ap()
    bvec = nc.dram_tensor("bvec", (128, 1), F32, kind="ExternalInput").ap()
    avec = nc.dram_tensor("avec", (64, 1), F32, kind="ExternalInput").ap()
    bevec = nc.dram_tensor("bevec", (64, 1), F32, kind="ExternalInput").ap()
    z = nc.dram_tensor("z", (ns, C, t_len), F32, kind="ExternalOutput").ap()

    W = ts + 8
    n_tiles = t_len // ts
    Tanh = mybir.ActivationFunctionType.Tanh
    Exp = mybir.ActivationFunctionType.Exp
    ADD = mybir.AluOpType.add

    with tile.TileContext(nc) as tc:
        with (
            tc.tile_pool(name="consts", bufs=1) as consts,
            tc.tile_pool(name="data", bufs=6) as data,
            tc.tile_pool(name="outs", bufs=6) as outs,
            tc.tile_pool(name="psum", bufs=4, space="PSUM") as psum_pool,
        ):
            wt_sb = consts.tile([128, 4, 128], F16)
            nc.sync.dma_start(wt_sb, wt.rearrange("m p o -> p m o"))
            bdd_sb = consts.tile([4, 128], F16)
            nc.sync.dma_start(bdd_sb, bdd)
            bind_sb = consts.tile([4, 512], F16)
            nc.sync.dma_start(bind_sb, bind)
            bvec_sb = consts.tile([128, 1], F32)
            nc.sync.dma_start(bvec_sb, bvec)
            bhi_sb = consts.tile([64, 1], F32)
            nc.sync.dma_start(bhi_sb, bvec[64:128, :])
            avec_sb = consts.tile([64, 1], F32)
            nc.sync.dma_start(avec_sb, avec)
            bevec_sb = consts.tile([64, 1], F32)
            nc.sync.dma_start(bevec_sb, bevec)

            for s in range(ns):
                for it in range(n_tiles):
                    t0 = it * ts
                    x2 = data.tile([128, W], F16, tag="x2", name="x2")
                    # partitions 0-63: x shifted 7; partitions 64-127: shifted 6
                    if it == 0:
                        nc.sync.dma_start(x2[0:64, 0:7], zpad[0:64, 0:7])
                        nc.sync.dma_start(x2[64:128, 0:6], zpad[64:128, 0:6])
                        nc.sync.dma_start(x2[0:64, 7:W], xs[s, :, 0:ts + 1])
                        nc.sync.dma_start(x2[64:128, 6:W], xs[s, :, 0:ts + 2])
                    elif it == n_tiles - 1:
                        nc.sync.dma_start(x2[0:64, 0:W - 1],
                                          xs[s, :, t0 - 7:t_len])
                        nc.sync.dma_start(x2[64:128, 0:W - 2],
                                          xs[s, :, t0 - 6:t_len])
                    else:
                        nc.sync.dma_start(x2[0:64, 0:W],
                                          xs[s, :, t0 - 7:t0 + ts + 1])
                        nc.sync.dma_start(x2[64:128, 0:W],
                                          xs[s, :, t0 - 6:t0 + ts + 2])

                    if USE_XF:
                        xt = data.tile([64, ts], F32, tag="xt", name="xt")
                        nc.sync.dma_start(xt, xf[s, :, t0:t0 + ts])

                    ps = psum_pool.tile([128, ts], F32, tag="ps", name="ps")
                    for h in range(ts // 512):
                        ho = h * 512
                        corr_here = (it == 0 and h == 0)
                        for m in range(4):
                            nc.tensor.matmul(
                                ps[:, ho:ho + 512],
                                wt_sb[:, m, :],
                                x2[:, ho + 2 * m:ho + 2 * m + 512],
                                start=(m == 0),
                                stop=(m == 3 and not corr_here))
                        if corr_here:
                            nc.tensor.matmul(
                                ps[:, 0:512], bdd_sb, bind_sb,
                                start=False, stop=True)

                    th = outs.tile([64, ts], F32, tag="th", name="th")
                    nc.scalar.activation(th, ps[0:64, :], Tanh,
                                         bias=bvec_sb[0:64, :])
                    e = outs.tile([64, ts], F32, tag="e", name="e")
                    nc.scalar.activation(e, th, Exp,
                                         bias=bevec_sb, scale=avec_sb)
                    zm = outs.tile([64, ts], F32, tag="zm", name="zm")
                    # alternate the multiply between DVE and GpSimd to keep
                    # the Vector engine (stt is pinned there) off the
                    # critical path
                    mul_eng = nc.vector if (it % 2 == 0) else nc.gpsimd
                    mul_eng.tensor_tensor(
                        zm, e, (xt if USE_XF else x2[0:64, 7:ts + 7]),
                        mybir.AluOpType.mult)
                    zt = outs.tile([64, ts], F32, tag="zt", name="zt")
                    nc.vector.scalar_tensor_tensor(
                        out=zt, in0=ps[64:128, :], scalar=bhi_sb,
                        in1=zm, op0=ADD, op1=ADD)
                    # z-store on the (otherwise idle) gpsimd SWDGE queue so it
                    # never head-of-line blocks the next tile's x2 loads on Sync
                    nc.gpsimd.dma_start(z[s, :, t0:t0 + ts], zt)

    nc.compile()
    return nc


def make_in_maps(x, w_in, b_in, w_mid, b_mid, w_out, b_out, alpha, beta,
                 n_cores=N_CORES):
    w_eff, b_eff, D = _compose(w_in, b_in, w_mid, b_mid, w_out, b_out)
    WT = _pack_weights(w_eff)
    BDD = np.ascontiguousarray(D.T)                   # (4, 128)
    BIND = np.zeros((4, 512), np.float32)
    for i in range(4):
        BIND[i, i] = 1.0
    bvec = np.ascontiguousarray(b_eff.reshape(128, 1))
    avec = np.ascontiguousarray(np.asarray(alpha, np.float32).reshape(64, 1))
    bevec = np.ascontiguousarray(np.asarray(beta, np.float32).reshape(64, 1))
    x = np.ascontiguousarray(np.asarray(x, np.float32))
    x16 = x.astype(np.float16)
    ns = x.shape[0] // n_cores
    maps = []
    for i in range(n_cores):
        m = dict(xs=np.ascontiguousarray(x16[i * ns:(i + 1) * ns]),
                 wt=WT.astype(np.float16), bdd=BDD.astype(np.float16),
                 bind=BIND.astype(np.float16),
                 zpad=np.zeros((128, 8), np.float16),
                 bvec=bvec, avec=avec, bevec=bevec)
        if USE_XF:
            m["xf"] = np.ascontiguousarray(x[i * ns:(i + 1) * ns])
        maps.append(m)
    return maps


_NC_CACHE = {}


def _get_nc():
    if "nc" not in _NC_CACHE:
        _NC_CACHE["nc"] = build_nc()
    return _NC_CACHE["nc"]


def kernel(x, w_in, b_in, w_mid, b_mid, w_out, b_out, alpha, beta,
           _trace=False, _trace_kwargs=None):
    nc = _get_nc()
    in_maps = make_in_maps(x, w_in, b_in, w_mid, b_mid, w_out, b_out,
                           alpha, beta)
    res = run_bass_kernel_spmd(nc, in_maps, core_ids=list(range(N_CORES)),
                               trace=_trace, **(_trace_kwargs or {}))
    out = np.concatenate([r["z"] for r in res.results], axis=0)
    kernel.last_results = res
    return out
